# revision 64
# baseline (speedup 1.0000x reference)
"""CapsuleLayer (dynamic routing) Trainium2 kernel.

Math (per example a):
  H[a,b,c,j] = sum_i x[a,c,i] * W[b,c,j,i]          (inputs_hat)
  3 routing iterations of:
    coef = softmax_b(L); s = sum_c coef*H; out = squash(s); L += sum_d out*H

Distribution: data-parallel over batch, 512 = 8 cores x 64 examples.

Per-core layout: SBUF partition p = b0*64 + a  (b0 = capsule_half, a = local
example); free axis of H is (b16, d16, c144), so every big routing op is a
2x-packed bf16 DVE/Pool op with c (or a c-pair) innermost.

H generation: all matmuls use k=32 tiles at partitions 0-31.  The x operand
is staged 5x: one unpadded copy (rows (r,i) = 8r+i) plus 4 r-masked copies
(zeros outside rows 8r..8r+8).  An H matmul for column group r uses the
masked variant, so the k=32 contraction picks out only that r's rows of the
shared W block; this keeps W in a single dense [32, ...] layout AND allows
s0 = sum_c H to be accumulated directly on the PE as 36 k=32 matmuls into a
dedicated PSUM bank (routing iteration 0 then needs no elementwise reduce).

PSUM banks hold a c-pair (two 256-wide groups, start=True/start=False), so
H drains are 72 512-wide copies round-robined over Pool/Act/DVE.

Cross-partition-half reductions (softmax denominator over both capsule
halves) use partition-offset operands directly - no SBUF DMA swap.
"""

import sys

for _p in ("/opt/trn_rl_repo",):
    if _p not in sys.path:
        sys.path.insert(0, _p)

from contextlib import ExitStack

import numpy as np

import concourse.bass as bass
import concourse.mybir as mybir
from concourse import tile
from concourse.bass_utils import run_bass_kernel_spmd

F32 = mybir.dt.float32
BF16 = mybir.dt.bfloat16
AF = mybir.ActivationFunctionType
ALU = mybir.AluOpType
AX = mybir.AxisListType

B = 512
NCORES = 8
BS = B // NCORES  # 64 examples per core
NCAP = 32
B16 = 16  # capsules per half
CIN = 144
CQ = 36  # c // 4
D = 16
I8 = 8
EPS = 1e-7
ROUTINGS = 3

ILB = CQ * BS  # 2304 columns per ilhs block
HFREE = B16 * D * CIN  # 36864 elements per partition
CSPL = 88  # c split: DVE owns [0, 88), Pool owns [88, 144)
CSPL2 = 88  # split for the final s-phase

HCH = 3  # cql per W chunk
NCH = CQ // HCH  # 12 chunks


def _build_program() -> bass.Bass:
    nc = bass.Bass()
    ilhs_d = nc.declare_dram_parameter("ilhs", [32, 5 * ILB], BF16, isOutput=False)
    wrhs_d = nc.declare_dram_parameter("wrhs", [32, CQ * 512], BF16, isOutput=False)
    sv0_d = nc.declare_dram_parameter("sv0", [128, 256], BF16, isOutput=True)
    sv1_d = nc.declare_dram_parameter("sv1", [128, 256], BF16, isOutput=True)

    V = nc.vector
    P = nc.gpsimd
    A = nc.scalar
    S = nc.sync

    with ExitStack() as ctx:
        tc = ctx.enter_context(tile.TileContext(nc))
        cpool = ctx.enter_context(tc.tile_pool(name="const", bufs=1))

        H = cpool.tile([128, HFREE], BF16)
        prod = cpool.tile([128, HFREE], BF16)
        ilhs_t = cpool.tile([32, 5 * ILB], BF16)

        s_t = cpool.tile([128, 256], F32)  # (b16, d)
        L_t = cpool.tile([128, B16 * CIN], F32)  # logits (b16, c)
        E_t = cpool.tile([128, B16 * CIN], BF16)
        C_t = cpool.tile([128, B16 * CIN], BF16)
        D2 = cpool.tile([128, CIN], BF16)
        Rh = cpool.tile([128, CIN], BF16)
        sq = cpool.tile([128, 256], F32)
        n2 = cpool.tile([128, 32], F32)
        rs = cpool.tile([128, 32], F32)
        t0 = cpool.tile([128, 32], F32)
        fac = cpool.tile([128, 32], F32)
        outB = cpool.tile([128, 256], BF16)
        outB2 = cpool.tile([128, 512], BF16)  # outB duplicated per c-pair
        epsb = cpool.tile([128, 1], F32)
        V.memset(epsb[:], EPS)

        # x operand blocks: 0 = unpadded, 1+r = masked to rows 8r..8r+8
        # r0/r1 variants first (the first matmuls use them), unpadded block
        # early on Pool (the first s0 matmul needs it)
        S.dma_start(ilhs_t[:, ILB : 2 * ILB], ilhs_d[:, ILB : 2 * ILB])
        A.dma_start(ilhs_t[:, 2 * ILB : 3 * ILB], ilhs_d[:, 2 * ILB : 3 * ILB])
        P.dma_start(ilhs_t[:, 0:ILB], ilhs_d[:, 0:ILB])

        Hv = H[:].rearrange("p (b d c) -> p b d c", b=B16, d=D)

        s0pool = ctx.enter_context(tc.tile_pool(name="s0p", bufs=1, space="PSUM"))
        s0psum = s0pool.tile([64, 512], F32)  # (b0, b16, d) on partitions a

        # ---- H generation + s0 accumulation ----
        # Emission schedule: W chunks prefetch 2 ahead; the s0 matmul of
        # chunk h is issued BEFORE the H matmuls of chunk h-1 (one-chunk
        # deferral), so the s0 accumulation - which gates routing iteration
        # 0 - finishes ~7us before the PE clears its H-matmul backlog.
        # squash0 is emitted between chunk 8's and chunk 9's drains so the
        # engines' queued work doesn't delay it; the last chunks drain on
        # Act alone (DVE/Pool must be free when s0 stops).
        dctr = [0]
        CHW = HCH * 512
        wtiles = {}

        # (defined below but emitted mid-loop)
        prodv = prod[:].rearrange("p (b d c) -> p b d c", b=B16, d=D)
        s_v = s_t[:].rearrange("p (b d) -> p b d", b=B16)
        L_v = L_t[:].rearrange("p (b c) -> p b c", b=B16)
        E_v = E_t[:].rearrange("p (b c) -> p b c", b=B16)
        C_v = C_t[:].rearrange("p (b c) -> p b c", b=B16)
        outB_v = outB[:].rearrange("p (b d) -> p b d", b=B16)
        outB2v = outB2[:].rearrange("p (bd c2) -> p bd c2", c2=2)
        prodbd = prod[:].rearrange("p (bd c) -> p bd c", c=CIN)
        Hbd = H[:].rearrange("p (bd c) -> p bd c", c=CIN)

        def d_sum(eng, sqv, n2dst):
            # n2dst[p, b] = sum_d sqv[p, b, d]; gpsimd lacks free-axis
            # tensor_reduce so fold d with an in-place tree instead
            if eng is V:
                eng.reduce_sum(n2dst, sqv, axis=AX.X)
                return
            for w in (8, 4, 2):
                eng.tensor_tensor(
                    sqv[:, :, 0:w], sqv[:, :, 0:w], sqv[:, :, w : 2 * w], op=ALU.add
                )
            eng.tensor_tensor(
                n2dst.unsqueeze(2), sqv[:, :, 0:1], sqv[:, :, 1:2], op=ALU.add
            )

        def squash0():
            # squash of s0 directly from PSUM, full-width on DVE alone (Pool
            # is still busy with W/drain work at this point, and a V->P
            # handoff would stall outB2).  s0psum = 32*s0; the 1/32 is folded
            # into the constants.  L_t is dead here and serves as scratch.
            scr = L_t[0:64, 0:512]
            # (a TensorTensor may read only ONE input from PSUM, so square
            # on Act instead)
            A.activation(scr, s0psum[:, :], AF.Square)
            V.reduce_sum(
                n2[0:64, :],
                scr.rearrange("p (b d) -> p b d", b=32),
                axis=AX.X,
            )
            # rs = sqrt(n2/1024 + eps) via ln+exp (one act table for all)
            A.activation(
                rs[0:64, :], n2[0:64, :], AF.Ln,
                bias=epsb[0:64], scale=1.0 / 1024.0,
            )
            A.activation(rs[0:64, :], rs[0:64, :], AF.Exp, scale=0.5)
            V.tensor_scalar(
                t0[0:64, :], n2[0:64, :], 1.0 / 1024.0, 1.0,
                op0=ALU.mult, op1=ALU.add,
            )
            V.tensor_tensor(t0[0:64, :], t0[0:64, :], rs[0:64, :], op=ALU.mult)
            V.reciprocal(t0[0:64, :], t0[0:64, :])
            V.tensor_tensor(fac[0:64, :], n2[0:64, :], t0[0:64, :], op=ALU.mult)
            for b0 in range(2):
                cs = slice(b0 * 256, (b0 + 1) * 256)
                bsl = slice(b0 * 16, (b0 + 1) * 16)
                facb = fac[0:64, bsl].unsqueeze(2).broadcast_to((64, 16, D))
                V.scalar_tensor_tensor(
                    outB_v[b0 * 64 : (b0 + 1) * 64, :, :],
                    s0psum[:, cs].rearrange("p (b d) -> p b d", b=16),
                    1.0 / 32768.0,
                    facb,
                    op0=ALU.mult,
                    op1=ALU.mult,
                )

        def squash():
            # split by capsule half across DVE/Pool; serial chains run in
            # parallel on the two engines
            for b0, eng in ((0, V), (1, P)):
                cs = slice(b0 * 128, (b0 + 1) * 128)
                bsl = slice(b0 * 8, (b0 + 1) * 8)
                eng.tensor_tensor(sq[:, cs], s_t[:, cs], s_t[:, cs], op=ALU.mult)
                d_sum(eng, sq[:, cs].rearrange("p (b d) -> p b d", b=8), n2[:, bsl])
                A.activation(rs[:, bsl], n2[:, bsl], AF.Ln, bias=epsb[:])
                A.activation(rs[:, bsl], rs[:, bsl], AF.Exp, scale=0.5)
                eng.tensor_scalar(
                    t0[:, bsl], n2[:, bsl], 1.0, None, op0=ALU.add
                )
                eng.tensor_tensor(t0[:, bsl], t0[:, bsl], rs[:, bsl], op=ALU.mult)
                # no divide in the TensorTensor ISA: reciprocal (DVE-only)
                # then multiply on the owning engine
                V.reciprocal(rs[:, bsl], t0[:, bsl])
                eng.tensor_tensor(
                    fac[:, bsl], n2[:, bsl], rs[:, bsl], op=ALU.mult
                )
                facb = (
                    fac[:, bsl].unsqueeze(2).broadcast_to((128, 8, D))
                )
                eng.tensor_tensor(
                    outB[:, cs].rearrange("p (b d) -> p b d", b=8),
                    s_v[:, bsl, :],
                    facb,
                    op=ALU.mult,
                )

        def b_update(first: bool):
            # prod = H * outB (expanded per c-pair to stay in DVE 2x mode),
            # then reduce over d and add into the logits.  On the first pass
            # Pool's multiply is split so its head starts before the last H
            # chunk has drained.
            V.tensor_copy(
                outB2v, outB[:].unsqueeze(2).broadcast_to((128, 256, 2))
            )
            def bu_mult(eng, mlo, mhi):
                npair = (mhi - mlo) // 2
                h_in = Hbd[:, :, mlo:mhi].rearrange(
                    "p bd (cp c2) -> p bd cp c2", c2=2
                )
                o_in = outB2v.unsqueeze(2).broadcast_to((128, 256, npair, 2))
                p_out = prodbd[:, :, mlo:mhi].rearrange(
                    "p bd (cp c2) -> p bd cp c2", c2=2
                )
                eng.tensor_tensor(p_out, h_in, o_in, op=ALU.mult)

            def bu_tree(eng, tlo, thi):
                for w in (8, 4):
                    eng.tensor_tensor(
                        prodv[:, :, 0:w, tlo:thi],
                        prodv[:, :, 0:w, tlo:thi],
                        prodv[:, :, w : 2 * w, tlo:thi],
                        op=ALU.add,
                    )
                # the last tree level and the logits add land in two halves
                # matching the softmax exp sub-ranges, so Act starts the
                # first exp earlier
                mid = (tlo + thi) // 2
                for lo, hi in ((tlo, mid), (mid, thi)):
                    eng.tensor_tensor(
                        prodv[:, :, 0:2, lo:hi],
                        prodv[:, :, 0:2, lo:hi],
                        prodv[:, :, 2:4, lo:hi],
                        op=ALU.add,
                    )
                    d0 = prodv[:, :, 0:1, lo:hi].squeeze(2)
                    d1 = prodv[:, :, 1:2, lo:hi].squeeze(2)
                    if first:
                        eng.tensor_tensor(L_v[:, :, lo:hi], d0, d1, op=ALU.add)
                    else:
                        eng.tensor_tensor(
                            L_v[:, :, lo:hi], L_v[:, :, lo:hi], d0, op=ALU.add
                        )
                        eng.tensor_tensor(
                            L_v[:, :, lo:hi], L_v[:, :, lo:hi], d1, op=ALU.add
                        )

            bu_mult(V, 0, CSPL)
            if first:
                # Pool streams behind the tail-chunk drains: multiply and
                # reduce the ready part of its range while the last chunk
                # (c >= 132) is still draining
                bu_mult(P, CSPL, 108)
                bu_mult(P, 108, 132)
                bu_tree(V, 0, CSPL)
                bu_tree(P, CSPL, 132)
                bu_mult(P, 132, CIN)
                bu_tree(P, 132, CIN)
            else:
                bu_mult(P, CSPL, CIN)
                bu_tree(V, 0, CSPL)
                bu_tree(P, CSPL, CIN)

        def softmax(last):
            # per-sub-range pipeline: exp (Act) -> b16 denominator tree ->
            # cross-half sum via partition-offset operands -> C = E/D ->
            # immediately start that sub-range's s-multiply for the NEXT
            # routing iteration, so the big multiplies begin well before the
            # whole softmax is done.  The final s-phase's tree split shifts
            # to CSPL2 to even out the two engines' finish times.
            Dt8 = C_t[:].rearrange("p (b c) -> p b c", b=B16)
            sp = CSPL2 if last else CSPL
            subs = [(V, 0, 44), (P, sp, 116), (V, 44, sp), (P, 116, CIN)]
            for eng, lo, hi in subs:
                A.activation(E_v[:, :, lo:hi], L_v[:, :, lo:hi], AF.Exp)
            cb = C_v.unsqueeze(2).broadcast_to((128, B16, D, CIN))
            for eng, lo, hi in subs:
                eng.tensor_tensor(
                    Dt8[:, 0:8, lo:hi], E_v[:, 0:8, lo:hi], E_v[:, 8:16, lo:hi],
                    op=ALU.add,
                )
                eng.tensor_tensor(
                    Dt8[:, 0:4, lo:hi], Dt8[:, 0:4, lo:hi], Dt8[:, 4:8, lo:hi],
                    op=ALU.add,
                )
                eng.tensor_tensor(
                    Dt8[:, 0:2, lo:hi], Dt8[:, 0:2, lo:hi], Dt8[:, 2:4, lo:hi],
                    op=ALU.add,
                )
                eng.tensor_tensor(
                    Dt8[:, 0:1, lo:hi],
                    Dt8[:, 0:1, lo:hi],
                    Dt8[:, 1:2, lo:hi],
                    op=ALU.add,
                )
                # both-halves denominator: a TensorTensor needs equal input
                # base partitions, but a partition-offset COPY is legal - so
                # copy the upper half down, add in place, duplicate back up
                eng.tensor_copy(D2[0:64, lo:hi], Dt8[64:128, 0, lo:hi])
                eng.tensor_tensor(
                    D2[0:64, lo:hi],
                    D2[0:64, lo:hi],
                    Dt8[0:64, 0, lo:hi],
                    op=ALU.add,
                )
                eng.tensor_copy(D2[64:128, lo:hi], D2[0:64, lo:hi])
                # no divide on the DVE ALU: DVE ranges use its reciprocal;
                # Pool ranges get 1/D = exp(-ln D) on Act so Pool's chain
                # never waits on DVE
                if eng is V:
                    with nc.allow_low_precision(
                        reason="softmax coefficients are bf16 throughout"
                    ):
                        V.reciprocal(Rh[:, lo:hi], D2[:, lo:hi])
                else:
                    A.activation(Rh[:, lo:hi], D2[:, lo:hi], AF.Ln)
                    A.activation(Rh[:, lo:hi], Rh[:, lo:hi], AF.Exp, scale=-1.0)
                rb = Rh[:, lo:hi].unsqueeze(1).broadcast_to((128, B16, hi - lo))
                eng.tensor_tensor(
                    C_v[:, :, lo:hi], E_v[:, :, lo:hi], rb, op=ALU.mult
                )
                eng.tensor_tensor(
                    prodv[:, :, :, lo:hi], Hv[:, :, :, lo:hi], cb[:, :, :, lo:hi],
                    op=ALU.mult,
                )

        def fold_range(eng, lo, hi):
            # halving add-tree over prod columns [lo, hi) into column lo
            w = hi - lo
            while w > 1:
                half = w // 2
                eng.tensor_tensor(
                    prodv[:, :, :, lo : lo + half],
                    prodv[:, :, :, lo : lo + half],
                    prodv[:, :, :, lo + half : lo + 2 * half],
                    op=ALU.add,
                )
                if w % 2 == 1:
                    eng.tensor_tensor(
                        prodv[:, :, :, lo : lo + 1],
                        prodv[:, :, :, lo : lo + 1],
                        prodv[:, :, :, lo + w - 1 : lo + w],
                        op=ALU.add,
                    )
                w = half

        def s_phase(split, last):
            # add tree over c per engine range (the C*H multiplies were issued
            # at the tail of the previous softmax)
            fold_range(V, 0, split)
            fold_range(P, split, CIN)
            if not last:
                # join halves into f32 s
                V.tensor_tensor(
                    s_v,
                    prodv[:, :, :, 0:1].squeeze(3),
                    prodv[:, :, :, split : split + 1].squeeze(3),
                    op=ALU.add,
                )
            else:
                # dump both tree columns; the host joins and squashes
                A.dma_start(
                    sv1_d[:, :],
                    prodv[:, :, :, split : split + 1].squeeze(3),
                )
                S.dma_start(sv0_d[:, :], prodv[:, :, :, 0:1].squeeze(3))

        with (
            tc.tile_pool(name="w", bufs=6) as wpool,
            tc.tile_pool(name="psum", bufs=7, space="PSUM") as pp,
        ):

            def issue_w(h):
                wc = wpool.tile([32, CHW], BF16)
                wtiles[h] = wc
                # SP (no compute duties) carries r0 and the early r3; the
                # late chunks (which gate the s0 stop) shift to Pool so they
                # don't queue behind SP's backlog
                weng = [P if h >= 10 else S, A, P, P if h >= 8 else S]
                for r in range(4):
                    weng[r].dma_start(
                        wc[8 * r : 8 * r + 8, :],
                        wrhs_d[8 * r : 8 * r + 8, h * CHW : (h + 1) * CHW],
                    )

            def h_mms(h):
                # H matmuls + paired-bank drains for chunk h
                wc = wtiles.pop(h)
                for cql in range(HCH):
                    cq = h * HCH + cql
                    for half in (0, 1):
                        pts = pp.tile([128, 512], F32, tag="pts")
                        for b0 in range(2):
                            for rr in range(2):
                                r = 2 * half + rr
                                lhs = ilhs_t[
                                    :,
                                    (1 + r) * ILB + cq * BS : (1 + r) * ILB
                                    + (cq + 1) * BS,
                                ]
                                rhs = wc[
                                    :,
                                    cql * 512 + b0 * 256 : cql * 512 + b0 * 256 + 256,
                                ]
                                nc.tensor.matmul(
                                    pts[
                                        b0 * 64 : (b0 + 1) * 64,
                                        rr * 256 : (rr + 1) * 256,
                                    ],
                                    lhs,
                                    rhs,
                                    start=(rr == 0),
                                    stop=(rr == 1),
                                    tile_position=(0, b0 * 64),
                                )
                        c0 = 4 * cq + 2 * half
                        dsts = Hv[:, :, :, c0 : c0 + 2]
                        srcs = pts[:].rearrange("p (c2 b d) -> p b d c2", c2=2, b=B16)
                        # GPSIMD cannot access PSUM, so only DVE and Act
                        # drain.  DVE must be free when s0 stops (its chunk-7
                        # share runs right after squash0); the tail chunks
                        # fall to Act alone.
                        if h < 7:
                            eng = [V, A, V, A][dctr[0] % 4]
                        elif h == 7:
                            eng = V
                        else:
                            eng = A
                        if eng is A:
                            eng.copy(dsts, srcs)
                        else:
                            eng.tensor_copy(dsts, srcs)
                        dctr[0] += 1

            issue_w(0)
            issue_w(1)
            for h in range(NCH):
                if h + 2 < NCH:
                    issue_w(h + 2)
                if h == 0:
                    # remaining x blocks, behind the first W chunk
                    P.dma_start(
                        ilhs_t[:, 3 * ILB : 4 * ILB], ilhs_d[:, 3 * ILB : 4 * ILB]
                    )
                    S.dma_start(
                        ilhs_t[:, 4 * ILB : 5 * ILB], ilhs_d[:, 4 * ILB : 5 * ILB]
                    )
                wc = wtiles[h]
                for cql in range(HCH):
                    cq = h * HCH + cql
                    nc.tensor.matmul(
                        s0psum[:, :],
                        ilhs_t[:, cq * BS : (cq + 1) * BS],
                        wc[:, cql * 512 : (cql + 1) * 512],
                        start=(cq == 0),
                        stop=(cq == CQ - 1),
                        tile_position=(0, 0),
                    )
                if h == NCH - 1:
                    # the accumulation group just closed; queue squash0 now so
                    # it runs ahead of the remaining drains
                    squash0()
                if 3 <= h <= 8:
                    h_mms(h - 3)
            for hh in range(6, NCH):
                h_mms(hh)

        b_update(first=True)
        softmax(last=False)
        s_phase(CSPL, last=False)
        squash()
        b_update(first=False)
        softmax(last=True)
        s_phase(CSPL2, last=True)

    # The TRN2 matmul ISA encoding only fits one sync wait; Tile can emit
    # several. Run the bacc fix-up passes: excess matmul waits move to the
    # paired ldweights, and any instruction still holding >1 wait gets them
    # split into preceding EventSemaphore instructions.
    import bass_rust as _bass_rust

    _bass_rust.move_matmul_waits_to_ldweights(nc.m)
    _bass_rust.generate_event_semaphores(nc)
    return nc


def _bf16(x: np.ndarray) -> np.ndarray:
    import ml_dtypes

    return x.astype(ml_dtypes.bfloat16)


def _pack_w(W: np.ndarray) -> np.ndarray:
    # wrhs[8r+i, cq*512 + b*16 + j] = W[b, 4cq+r, j, i]
    wrhs = np.empty((32, CQ * 512), np.float32)
    for r in range(4):
        blk = W[:, r::4, :, :]  # [b, cq, j, i]
        wrhs[8 * r : 8 * r + 8, :] = np.ascontiguousarray(
            blk.transpose(3, 1, 0, 2)
        ).reshape(8, CQ * 512)
    return _bf16(wrhs)


def _pack_x(xs: np.ndarray) -> np.ndarray:
    # block 0 (unpadded): ilhs[8r+i, cq*64 + a] = xs[a, 4cq+r, i]
    # block 1+r: same rows 8r..8r+8, zero elsewhere
    base = np.zeros((32, ILB), np.float32)
    for r in range(4):
        blk = xs[:, r::4, :]  # [a, cq, i]
        base[8 * r : 8 * r + 8, :] = np.ascontiguousarray(
            blk.transpose(2, 1, 0)
        ).reshape(8, ILB)
    ilhs = np.zeros((32, 5 * ILB), np.float32)
    ilhs[:, 0:ILB] = base
    for r in range(4):
        ilhs[8 * r : 8 * r + 8, (1 + r) * ILB : (2 + r) * ILB] = base[
            8 * r : 8 * r + 8, :
        ]
    return _bf16(ilhs)


_CACHED = {}


def _get_program():
    if "nc" not in _CACHED:
        _CACHED["nc"] = _build_program()
    return _CACHED["nc"]


def kernel(inputs: np.ndarray, W: np.ndarray) -> np.ndarray:
    inputs = np.asarray(inputs, np.float32)
    W = np.asarray(W, np.float32)
    nc = _get_program()
    wrhs = _pack_w(W)
    in_maps = []
    for k in range(NCORES):
        xs = inputs[k * BS : (k + 1) * BS]
        in_maps.append({"ilhs": _pack_x(xs), "wrhs": wrhs})
    res = run_bass_kernel_spmd(nc, in_maps, core_ids=list(range(NCORES)))
    outs = []
    for k in range(NCORES):
        s = res.results[k]["sv0"].astype(np.float32) + res.results[k][
            "sv1"
        ].astype(np.float32)
        # partitions (b0, a) x free (b16, d) -> [a, b, d], then squash
        s = s.reshape(2, BS, B16, D).transpose(1, 0, 2, 3).reshape(BS, NCAP, D)
        n2 = np.sum(s * s, axis=-1, keepdims=True)
        outs.append((n2 / (1.0 + n2)) * s / np.sqrt(n2 + EPS))
    return np.concatenate(outs, axis=0).astype(np.float32)
